# revision 11
# baseline (speedup 1.0000x reference)
"""Trainium2 Bass kernel for nn_LocalMean: 5x5 box filter, reflect padding.

Input:  image [16, 3, 1024, 1024] fp32
Output: same shape; out[h,w] = mean of 5x5 reflect-padded window.

Strategy (pure data parallel, 8 cores, 2 images/core = 6 planes of 1024^2):
  - Horizontal pass: running-window sum via DVE tensor_tensor_scan
      r[w] = r[w-1] + x[w+2] - x[w-3]   (reflect cols materialized in SBUF)
  - Vertical pass: banded fp32 matmul  out = B.T @ r  with reflect weights
      and the 1/25 scale folded into B.
  - PSUM -> SBUF copies on ScalarE; loads on sync-HWDGE, stores on ACT-HWDGE.
  - Row tiling: 9 output tiles of 124 rows (last 32); input tiles overlap
    by 4 rows so each output tile needs exactly one input tile (<=128 rows).
"""

import numpy as np

N_CORES = 8
PLANES = 6            # 2 images x 3 channels per core
H = W = 1024
PATCH = 5
PAD = 2
OUT_TILE = 124        # output rows per tile (input rows = 124 + 4 <= 128)
N_TILES = 9           # 8 * 124 + 32 = 1024
BLK = 1036            # per-plane column stride in the padded SBUF tile
XCOLS = PLANES * BLK  # padded tile width
SCAN_N = W + PATCH    # scan runs 5 extra warm-up iterations from state=0
RBLK = 1032           # per-plane column stride in the r tile (1029 padded)
RCOLS = PLANES * RBLK


def _reflect(r):
    if r < 0:
        return -r
    if r > H - 1:
        return 2 * (H - 1) - r
    return r


def _tile_geometry(t):
    """Returns (in_row0, K, out_row0, M) for row-tile t."""
    r0 = t * OUT_TILE - PAD
    r0c = max(r0, 0)
    r1 = min(r0 + OUT_TILE + 2 * PAD, H)
    K = r1 - r0c
    out_row0 = t * OUT_TILE
    M = min(OUT_TILE, H - out_row0)
    return r0c, K, out_row0, M


def _build_B(t):
    """Banded vertical-window matrix for tile t: B[k, m] = (1/25) * mult of
    input row (in_row0 + k) in the reflected window of output row
    (out_row0 + m)."""
    r0c, K, out_row0, M = _tile_geometry(t)
    B = np.zeros((K, M), np.float32)
    for m in range(M):
        for d in range(-PAD, PAD + 1):
            rr = _reflect(out_row0 + m + d)
            k = rr - r0c
            assert 0 <= k < K, (t, m, d, rr, r0c, K)
            B[k, m] += 1.0
    return B * np.float32(1.0 / (PATCH * PATCH))


def _build_module():
    import concourse.bacc as bacc
    import concourse.mybir as mybir
    from concourse.tile import TileContext

    f32 = mybir.dt.float32
    nc = bacc.Bacc(trn_type="TRN2")

    x = nc.dram_tensor("x", [PLANES, H, W], f32, kind="ExternalInput")
    y = nc.dram_tensor("y", [PLANES, H, W], f32, kind="ExternalOutput")

    # Three distinct banded matrices: top (reflect), interior, bottom (reflect)
    B_np = {0: _build_B(0), 1: _build_B(1), 8: _build_B(8)}
    for t in range(2, 8):
        assert np.array_equal(_build_B(t), B_np[1])
    B_dram = {k: nc.inline_tensor(v, name=f"Bmat{k}") for k, v in B_np.items()}

    with TileContext(nc) as tc:
        with tc.tile_pool(name="consts", bufs=1) as cpool, \
             tc.tile_pool(name="xpad", bufs=2) as xpool, \
             tc.tile_pool(name="rsum", bufs=3) as rpool, \
             tc.tile_pool(name="init", bufs=2) as ipool, \
             tc.tile_pool(name="outs", bufs=2) as opool, \
             tc.tile_pool(name="psum", bufs=8, space="PSUM") as pspool:

            B_tiles = {}
            for key, dram in B_dram.items():
                kk, mm = B_np[key].shape
                bt = cpool.tile([128, mm], f32, tag=f"B{key}")
                nc.sync.dma_start(out=bt[:kk, :], in_=dram[:, :])
                B_tiles[key] = bt

            for t in range(N_TILES):
                r0c, K, out_row0, M = _tile_geometry(t)
                b_key = 0 if t == 0 else (8 if t == 8 else 1)
                bt = B_tiles[b_key]

                xp = xpool.tile([128, XCOLS], f32, tag="xp")
                xp3 = xp[:K].rearrange("k (p c) -> k p c", c=BLK)
                # col j holds padded x[j-8]: j 0..5 zeros, 6 -> x[2],
                # 7 -> x[1], 8..1031 -> x[0..1023], 1032 -> x[1022],
                # 1033 -> x[1021]
                nc.sync.dma_start(
                    out=xp3[:, :, 8:8 + W],
                    in_=x[:, r0c:r0c + K, :].rearrange("p r c -> r p c"),
                )
                nc.vector.memset(xp3[:, :, 0:6], 0.0)
                nc.scalar.copy(out=xp3[:, :, 6:7], in_=xp3[:, :, 10:11])
                nc.scalar.copy(out=xp3[:, :, 7:8], in_=xp3[:, :, 9:10])
                nc.scalar.copy(out=xp3[:, :, 1032:1033],
                               in_=xp3[:, :, 1030:1031])
                nc.scalar.copy(out=xp3[:, :, 1033:1034],
                               in_=xp3[:, :, 1029:1030])

                rt = rpool.tile([128, RCOLS], f32, tag="rt")
                ot = opool.tile([128, PLANES * W], f32, tag="ot")

                for p in range(PLANES):
                    # r[w] = r[w-1] + xpad[w+2] - xpad[w-3], w = -5..1023,
                    # from state 0 (the first 5 outputs are warm-up).
                    nc.vector.tensor_tensor_scan(
                        out=rt[:K, p * RBLK:p * RBLK + SCAN_N],
                        data0=xp[:K, p * BLK + 5:p * BLK + 5 + SCAN_N],
                        data1=xp[:K, p * BLK:p * BLK + SCAN_N],
                        initial=0.0,
                        op0=mybir.AluOpType.add,
                        op1=mybir.AluOpType.subtract,
                    )
                    for h in range(2):
                        ps = pspool.tile([128, 512], f32, tag="ps")
                        nc.tensor.matmul(
                            ps[:M, :], bt[:K, :M],
                            rt[:K, p * RBLK + 5 + h * 512:
                                p * RBLK + 5 + (h + 1) * 512],
                            start=True, stop=True,
                        )
                        nc.scalar.copy(
                            out=ot[:M, p * W + h * 512:p * W + (h + 1) * 512],
                            in_=ps[:M, :],
                        )

                nc.gpsimd.dma_start(
                    out=y[:, out_row0:out_row0 + M, :].rearrange("p r c -> r p c"),
                    in_=ot[:M].rearrange("m (p c) -> m p c", c=W),
                )

    nc.finalize()
    return nc


_NC = None


def _get_nc():
    global _NC
    if _NC is None:
        _NC = _build_module()
    return _NC


def _run_spmd(image, trace=False):
    from concourse import bass_utils

    image = np.ascontiguousarray(np.asarray(image, dtype=np.float32))
    assert image.shape == (16, 3, H, W), image.shape
    in_maps = [
        {"x": image[2 * c:2 * c + 2].reshape(PLANES, H, W)}
        for c in range(N_CORES)
    ]
    nc = _get_nc()
    res = bass_utils.run_bass_kernel_spmd(
        nc, in_maps, core_ids=list(range(N_CORES)), trace=trace,
    )
    out = np.concatenate(
        [res.results[c]["y"].reshape(2, 3, H, W) for c in range(N_CORES)],
        axis=0,
    )
    return out, res


def kernel(image):
    out, _ = _run_spmd(image, trace=False)
    return out


# revision 13
# speedup vs baseline: 1.4530x; 1.4530x over previous
"""Trainium2 Bass kernel for nn_LocalMean: 5x5 box filter, reflect padding.

Input:  image [16, 3, 1024, 1024] fp32
Output: same shape; out[h,w] = mean of 5x5 reflect-padded window.

Strategy (pure data parallel, 8 cores, 2 images/core = 6 planes of 1024^2):
  - Horizontal pass: running-window sum via DVE tensor_tensor_scan
      r[w] = r[w-1] + x[w+2] - x[w-3]   (reflect cols materialized in SBUF)
  - Vertical pass: banded fp32 matmul  out = B.T @ r  with reflect weights
      and the 1/25 scale folded into B.
  - PSUM -> SBUF copies on ScalarE; loads on sync-HWDGE, stores on ACT-HWDGE.
  - Row tiling: 9 output tiles of 124 rows (last 32); input tiles overlap
    by 4 rows so each output tile needs exactly one input tile (<=128 rows).
"""

import numpy as np

N_CORES = 8
PLANES = 6            # 2 images x 3 channels per core
H = W = 1024
PATCH = 5
PAD = 2
OUT_TILE = 124        # output rows per tile (input rows = 124 + 4 <= 128)
N_TILES = 9           # 8 * 124 + 32 = 1024
BLK = 1036            # per-plane column stride in the padded SBUF tile
XCOLS = PLANES * BLK  # padded tile width
SCAN_N = W + PATCH    # scan runs 5 extra warm-up iterations from state=0
RBLK = 1032           # per-plane column stride in the r tile (1029 padded)
RCOLS = PLANES * RBLK


def _reflect(r):
    if r < 0:
        return -r
    if r > H - 1:
        return 2 * (H - 1) - r
    return r


def _tile_geometry(t):
    """Returns (in_row0, K, out_row0, M) for row-tile t."""
    r0 = t * OUT_TILE - PAD
    r0c = max(r0, 0)
    r1 = min(r0 + OUT_TILE + 2 * PAD, H)
    K = r1 - r0c
    out_row0 = t * OUT_TILE
    M = min(OUT_TILE, H - out_row0)
    return r0c, K, out_row0, M


def _build_B(t):
    """Banded vertical-window matrix for tile t: B[k, m] = (1/25) * mult of
    input row (in_row0 + k) in the reflected window of output row
    (out_row0 + m)."""
    r0c, K, out_row0, M = _tile_geometry(t)
    B = np.zeros((K, M), np.float32)
    for m in range(M):
        for d in range(-PAD, PAD + 1):
            rr = _reflect(out_row0 + m + d)
            k = rr - r0c
            assert 0 <= k < K, (t, m, d, rr, r0c, K)
            B[k, m] += 1.0
    return B * np.float32(1.0 / (PATCH * PATCH))


def _build_module():
    import concourse.bacc as bacc
    import concourse.mybir as mybir
    from concourse.tile import TileContext

    f32 = mybir.dt.float32
    nc = bacc.Bacc(trn_type="TRN2")

    x = nc.dram_tensor("x", [PLANES, H, W], f32, kind="ExternalInput")
    y = nc.dram_tensor("y", [PLANES, H, W], f32, kind="ExternalOutput")

    # Three distinct banded matrices: top (reflect), interior, bottom (reflect)
    B_np = {0: _build_B(0), 1: _build_B(1), 8: _build_B(8)}
    for t in range(2, 8):
        assert np.array_equal(_build_B(t), B_np[1])
    B_dram = {k: nc.inline_tensor(v, name=f"Bmat{k}") for k, v in B_np.items()}

    with TileContext(nc) as tc:
        with tc.tile_pool(name="consts", bufs=1) as cpool, \
             tc.tile_pool(name="xpad", bufs=2) as xpool, \
             tc.tile_pool(name="rsum", bufs=4) as rpool, \
             tc.tile_pool(name="outs", bufs=8) as opool, \
             tc.tile_pool(name="psum", bufs=8, space="PSUM") as pspool:

            B_tiles = {}
            for key, dram in B_dram.items():
                kk, mm = B_np[key].shape
                bt = cpool.tile([128, mm], f32, tag=f"B{key}")
                nc.sync.dma_start(out=bt[:kk, :], in_=dram[:, :])
                B_tiles[key] = bt

            def load_tile(t):
                r0c, K, _, _ = _tile_geometry(t)
                xp = xpool.tile([128, XCOLS], f32, tag="xp")
                xp3 = xp[:K].rearrange("k (p c) -> k p c", c=BLK)
                # col j holds padded x[j-8]: j 0..5 zeros, 6 -> x[2],
                # 7 -> x[1], 8..1031 -> x[0..1023], 1032 -> x[1022],
                # 1033 -> x[1021]
                nc.sync.dma_start(
                    out=xp3[:, :, 8:8 + W],
                    in_=x[:, r0c:r0c + K, :].rearrange("p r c -> r p c"),
                )
                nc.vector.memset(xp3[:, :, 0:6], 0.0)
                nc.scalar.copy(out=xp3[:, :, 6:7], in_=xp3[:, :, 10:11])
                nc.scalar.copy(out=xp3[:, :, 7:8], in_=xp3[:, :, 9:10])
                nc.scalar.copy(out=xp3[:, :, 1032:1033],
                               in_=xp3[:, :, 1030:1031])
                nc.scalar.copy(out=xp3[:, :, 1033:1034],
                               in_=xp3[:, :, 1029:1030])
                return xp

            xps = {0: load_tile(0)}
            for t in range(N_TILES):
                r0c, K, out_row0, M = _tile_geometry(t)
                b_key = 0 if t == 0 else (8 if t == 8 else 1)
                bt = B_tiles[b_key]
                if t + 1 < N_TILES:
                    xps[t + 1] = load_tile(t + 1)
                xp = xps.pop(t)

                rt = rpool.tile([128, RCOLS], f32, tag="rt")

                for p in range(PLANES):
                    # r[w] = r[w-1] + xpad[w+2] - xpad[w-3], w = -5..1023,
                    # from state 0 (the first 5 outputs are warm-up).
                    nc.vector.tensor_tensor_scan(
                        out=rt[:K, p * RBLK:p * RBLK + SCAN_N],
                        data0=xp[:K, p * BLK + 5:p * BLK + 5 + SCAN_N],
                        data1=xp[:K, p * BLK:p * BLK + SCAN_N],
                        initial=0.0,
                        op0=mybir.AluOpType.add,
                        op1=mybir.AluOpType.subtract,
                    )
                    stage = opool.tile([128, W], f32, tag="stage")
                    for h in range(2):
                        ps = pspool.tile([128, 512], f32, tag="ps")
                        nc.tensor.matmul(
                            ps[:M, :], bt[:K, :M],
                            rt[:K, p * RBLK + 5 + h * 512:
                                p * RBLK + 5 + (h + 1) * 512],
                            start=True, stop=True,
                        )
                        nc.scalar.copy(
                            out=stage[:M, h * 512:(h + 1) * 512],
                            in_=ps[:M, :],
                        )
                    nc.gpsimd.dma_start(
                        out=y[p, out_row0:out_row0 + M, :],
                        in_=stage[:M, :],
                    )

    nc.finalize()
    return nc


_NC = None


def _get_nc():
    global _NC
    if _NC is None:
        _NC = _build_module()
    return _NC


def _run_spmd(image, trace=False):
    from concourse import bass_utils

    image = np.ascontiguousarray(np.asarray(image, dtype=np.float32))
    assert image.shape == (16, 3, H, W), image.shape
    in_maps = [
        {"x": image[2 * c:2 * c + 2].reshape(PLANES, H, W)}
        for c in range(N_CORES)
    ]
    nc = _get_nc()
    res = bass_utils.run_bass_kernel_spmd(
        nc, in_maps, core_ids=list(range(N_CORES)), trace=trace,
    )
    out = np.concatenate(
        [res.results[c]["y"].reshape(2, 3, H, W) for c in range(N_CORES)],
        axis=0,
    )
    return out, res


def kernel(image):
    out, _ = _run_spmd(image, trace=False)
    return out


# revision 14
# speedup vs baseline: 1.6277x; 1.1202x over previous
"""Trainium2 Bass kernel for nn_LocalMean: 5x5 box filter, reflect padding.

Input:  image [16, 3, 1024, 1024] fp32
Output: same shape; out[h,w] = mean of 5x5 reflect-padded window.

Strategy (pure data parallel, 8 cores, 2 images/core = 6 planes of 1024^2):
  - Horizontal pass: running-window sum via DVE tensor_tensor_scan
      r[w] = r[w-1] + x[w+2] - x[w-3]   (reflect cols materialized in SBUF)
  - Vertical pass: banded fp32 matmul  out = B.T @ r  with reflect weights
      and the 1/25 scale folded into B.
  - PSUM -> SBUF copies on ScalarE; loads on sync-HWDGE, stores on ACT-HWDGE.
  - Row tiling: 9 output tiles of 124 rows (last 32); input tiles overlap
    by 4 rows so each output tile needs exactly one input tile (<=128 rows).
"""

import numpy as np

N_CORES = 8
PLANES = 6            # 2 images x 3 channels per core
H = W = 1024
PATCH = 5
PAD = 2
OUT_TILE = 124        # output rows per tile (input rows = 124 + 4 <= 128)
N_TILES = 9           # 8 * 124 + 32 = 1024
BLK = 1036            # per-plane column stride in the padded SBUF tile
XCOLS = PLANES * BLK  # padded tile width
SCAN_N = W + PATCH    # scan runs 5 extra warm-up iterations from state=0
RBLK = 1032           # per-plane column stride in the r tile (1029 padded)
RCOLS = PLANES * RBLK


def _reflect(r):
    if r < 0:
        return -r
    if r > H - 1:
        return 2 * (H - 1) - r
    return r


def _tile_geometry(t):
    """Returns (in_row0, K, out_row0, M) for row-tile t."""
    r0 = t * OUT_TILE - PAD
    r0c = max(r0, 0)
    r1 = min(r0 + OUT_TILE + 2 * PAD, H)
    K = r1 - r0c
    out_row0 = t * OUT_TILE
    M = min(OUT_TILE, H - out_row0)
    return r0c, K, out_row0, M


def _build_B(t):
    """Banded vertical-window matrix for tile t: B[k, m] = (1/25) * mult of
    input row (in_row0 + k) in the reflected window of output row
    (out_row0 + m)."""
    r0c, K, out_row0, M = _tile_geometry(t)
    B = np.zeros((K, M), np.float32)
    for m in range(M):
        for d in range(-PAD, PAD + 1):
            rr = _reflect(out_row0 + m + d)
            k = rr - r0c
            assert 0 <= k < K, (t, m, d, rr, r0c, K)
            B[k, m] += 1.0
    return B * np.float32(1.0 / (PATCH * PATCH))


def _build_module():
    import concourse.bacc as bacc
    import concourse.mybir as mybir
    from concourse.tile import TileContext

    f32 = mybir.dt.float32
    nc = bacc.Bacc(trn_type="TRN2")

    x = nc.dram_tensor("x", [PLANES, H, W], f32, kind="ExternalInput")
    y = nc.dram_tensor("y", [PLANES, H, W], f32, kind="ExternalOutput")

    # Three distinct banded matrices: top (reflect), interior, bottom (reflect)
    B_np = {0: _build_B(0), 1: _build_B(1), 8: _build_B(8)}
    for t in range(2, 8):
        assert np.array_equal(_build_B(t), B_np[1])
    B_dram = {k: nc.inline_tensor(v, name=f"Bmat{k}") for k, v in B_np.items()}

    with TileContext(nc) as tc:
        with tc.tile_pool(name="consts", bufs=1) as cpool, \
             tc.tile_pool(name="xpad", bufs=3) as xpool, \
             tc.tile_pool(name="rsum", bufs=3) as rpool, \
             tc.tile_pool(name="outs", bufs=8) as opool, \
             tc.tile_pool(name="psum", bufs=8, space="PSUM") as pspool:

            B_tiles = {}
            for key, dram in B_dram.items():
                kk, mm = B_np[key].shape
                bt = cpool.tile([128, mm], f32, tag=f"B{key}")
                nc.sync.dma_start(out=bt[:kk, :], in_=dram[:, :])
                B_tiles[key] = bt

            def load_tile(t):
                r0c, K, _, _ = _tile_geometry(t)
                xp = xpool.tile([128, XCOLS], f32, tag="xp")
                xp3 = xp[:K].rearrange("k (p c) -> k p c", c=BLK)
                # col j holds padded x[j-8]: j 0..5 zeros, 6 -> x[2],
                # 7 -> x[1], 8..1031 -> x[0..1023], 1032 -> x[1022],
                # 1033 -> x[1021]
                nc.sync.dma_start(
                    out=xp3[:, :, 8:8 + W],
                    in_=x[:, r0c:r0c + K, :].rearrange("p r c -> r p c"),
                )
                nc.vector.memset(xp3[:, :, 0:6], 0.0)
                nc.scalar.copy(out=xp3[:, :, 6:7], in_=xp3[:, :, 10:11])
                nc.scalar.copy(out=xp3[:, :, 7:8], in_=xp3[:, :, 9:10])
                nc.scalar.copy(out=xp3[:, :, 1032:1033],
                               in_=xp3[:, :, 1030:1031])
                nc.scalar.copy(out=xp3[:, :, 1033:1034],
                               in_=xp3[:, :, 1029:1030])
                return xp

            xps = {0: load_tile(0), 1: load_tile(1)}
            for t in range(N_TILES):
                r0c, K, out_row0, M = _tile_geometry(t)
                b_key = 0 if t == 0 else (8 if t == 8 else 1)
                bt = B_tiles[b_key]
                if t + 2 < N_TILES:
                    xps[t + 2] = load_tile(t + 2)
                xp = xps.pop(t)

                rt = rpool.tile([128, RCOLS], f32, tag="rt")

                for p in range(PLANES):
                    # r[w] = r[w-1] + xpad[w+2] - xpad[w-3], w = -5..1023,
                    # from state 0 (the first 5 outputs are warm-up).
                    nc.vector.tensor_tensor_scan(
                        out=rt[:K, p * RBLK:p * RBLK + SCAN_N],
                        data0=xp[:K, p * BLK + 5:p * BLK + 5 + SCAN_N],
                        data1=xp[:K, p * BLK:p * BLK + SCAN_N],
                        initial=0.0,
                        op0=mybir.AluOpType.add,
                        op1=mybir.AluOpType.subtract,
                    )
                    stage = opool.tile([128, W], f32, tag="stage")
                    for h in range(2):
                        ps = pspool.tile([128, 512], f32, tag="ps")
                        nc.tensor.matmul(
                            ps[:M, :], bt[:K, :M],
                            rt[:K, p * RBLK + 5 + h * 512:
                                p * RBLK + 5 + (h + 1) * 512],
                            start=True, stop=True,
                        )
                        nc.scalar.copy(
                            out=stage[:M, h * 512:(h + 1) * 512],
                            in_=ps[:M, :],
                        )
                    nc.gpsimd.dma_start(
                        out=y[p, out_row0:out_row0 + M, :],
                        in_=stage[:M, :],
                    )

    nc.finalize()
    return nc


_NC = None


def _get_nc():
    global _NC
    if _NC is None:
        _NC = _build_module()
    return _NC


def _run_spmd(image, trace=False):
    from concourse import bass_utils

    image = np.ascontiguousarray(np.asarray(image, dtype=np.float32))
    assert image.shape == (16, 3, H, W), image.shape
    in_maps = [
        {"x": image[2 * c:2 * c + 2].reshape(PLANES, H, W)}
        for c in range(N_CORES)
    ]
    nc = _get_nc()
    res = bass_utils.run_bass_kernel_spmd(
        nc, in_maps, core_ids=list(range(N_CORES)), trace=trace,
    )
    out = np.concatenate(
        [res.results[c]["y"].reshape(2, 3, H, W) for c in range(N_CORES)],
        axis=0,
    )
    return out, res


def kernel(image):
    out, _ = _run_spmd(image, trace=False)
    return out
